# revision 14
# baseline (speedup 1.0000x reference)
"""Trainium2 Bass kernel for unscaled dot-product attention.

Shapes (hardcoded): query/key/value [2048, 2, 16, 64] fp32.
  scores = einsum('sbnh,tbnh->bnst', q, k)   (UNscaled)
  probs  = softmax(scores, axis=-1)
  out    = einsum('bnst,tbnh->sbnh', probs, v).reshape(2048, 2, 1024)

Sharding: the 32 (b, n) head-slices are split 4-per-core across 8 cores
(core c -> b = c//4, heads 4*(c%4) .. +4). Each core computes attention
for its 4 heads independently; no cross-device communication.

Device-side strategy (per core, heads processed in 2 pairs):
  - Inputs arrive as separate K^T / Q^T / V' DRAM tensors, DMA-ed in
    consumption order (first K/Q/V chunks of pair 0 first) so the first
    QK matmul can start as soon as ~1.8MB has landed instead of waiting
    for the full 6.4MB.
  - Q is pre-scaled by log2(e) on the host, so on-device scores are
    t = score*log2(e); exp is computed as 2^t (ACT runs Exp with
    scale=ln2, the custom DVE path computes 2^t directly).
  - scores are computed TRANSPOSED per 128-t block with two heads packed
    into the 128 PE contraction rows (row-tiled concurrent matmuls).
  - Normalization happens ON THE HOST: the device ships the unnormalized
    context^T (PV accumulation, with a ones column producing the softmax
    denominator in row 64) straight from PSUM to DRAM; the host divides
    and transposes. This removes reciprocal/broadcast/multiply work from
    the device entirely.
  - exp() is split between the Scalar engine (ACT spline Exp) and the
    Vector engine (custom 8-stage DVE op computing the 2^t bit pattern:
    magic-rounding range reduction + quadratic mantissa correction,
    finished by a GPSIMD affine pass + f32->int32 convert that builds
    the final float bits).
"""

import numpy as np

SQ, B, NHEADS, HN = 2048, 2, 16, 64
N_CORES = 8
HEADS_PER_CORE = 4
VW = 66                     # V' columns per head (64 V + ones + pad)
LOG2E = 1.4426950408889634
LN2 = 0.6931471805599453

SCH = 512                   # s-chunk per inner loop
NCH = SQ // SCH             # 4
NT = SQ // 128              # 16 t-blocks

# custom-DVE exp2 pass-1 constants (see _register_exp2_op)
EXP_C0 = 126.5
EXP_C1 = 1.5 * 2.0**23
EXP_B2 = 0.333205057        # minimax quadratic coeff, lambda=1 kink-free
# pass-2 bias (centered for truncation; a round-mode instead just shifts the
# global scale, which softmax normalization cancels)
EXP_BETA0 = 0.415565974 + 0.5 / 128

_CACHE = {}


def _register_exp2_op():
    """Register the EXP2_PASS1_ANT custom DVE op (runtime extension of
    concourse.dve_ops via its documented OPS registry).

    Computes, per element (t = score * log2(e), from Src0):
      i+127 = magic-round(t + 126.5)      (floor(t) + 127)
      h     = frac(t) - 0.5
      v1    = (i + 127) + h + b2*h^2      (8 ALU stages exactly)
    A second pass ((v1 + beta0) * 2^23 -> int32) then forms the IEEE-754
    bit pattern of ~2^t.  Max relative error ~2.9e-3, global scale 1
    (cancels in softmax normalization anyway).
    """
    import concourse.dve_ops as dve_ops
    for op in dve_ops.OPS:
        if op.name == "EXP2_PASS1_ANT":
            return op
    from concourse.dve_spec import Spec, Src0, C0, C1, C2, One, lower
    from concourse.dve_spec import _has_src1
    from concourse.dve_uop import DveOpSpec

    pre = Src0 + C0
    u = pre + C1
    w = u - C1
    s = pre - w
    h2 = (s * C2) + One
    g = s * h2
    body = w + g

    def ref(in0, in1, s0, s1, imm2):
        f32 = np.float32
        t = np.asarray(in0, f32)
        pre = (t + f32(s0)).astype(f32)
        u = (pre + f32(s1)).astype(f32)
        w = (u - f32(s1)).astype(f32)
        sh = (pre - w).astype(f32)
        hh2 = ((sh * f32(imm2)).astype(f32) + f32(1.0)).astype(f32)
        g = (sh * hh2).astype(f32)
        return (w + g).astype(f32)

    spec = Spec(body=body, reference=ref)
    row = dve_ops._CUSTOM_DVE_ROW_BASE + len(dve_ops.OPS)
    sha = {}
    for ver in ("v3", "v4"):
        sha[ver] = DveOpSpec(
            name="EXP2_PASS1_ANT", opcode=row, uops=lower(spec, ver=ver),
            rd1_en=_has_src1(spec)).sha(ver)
    op = dve_ops.DveOp("EXP2_PASS1_ANT", spec, subdim=False, uops_sha=sha)
    dve_ops.OPS.append(op)
    dve_ops._SUB_OPCODE_FOR_NAME[op.name] = row
    dve_ops.CUSTOM_DVE_SPECS[op.name] = spec
    return op


def _round_fp32r(x):
    """Round fp32 array to the fp32r grid (11 explicit mantissa bits,
    round-to-nearest-even, low 12 bits zero)."""
    u = np.ascontiguousarray(x, np.float32).view(np.uint32)
    lsb = (u >> 12) & 1
    u = (u + 0x7FF + lsb) & 0xFFFFF000
    return u.astype(np.uint32).view(np.float32)


def _build_program(dve_every=0):
    """dve_every=0: all exp on ACT.  dve_every=k>0: every k-th step's exp
    runs on the DVE+GPSIMD path instead."""
    from contextlib import ExitStack

    import concourse.bacc as bacc
    import concourse.mybir as mybir
    import concourse.tile as tile

    f32 = mybir.dt.float32
    f32r = mybir.dt.float32r
    bf16 = mybir.dt.bfloat16
    i16 = mybir.dt.int16
    EXP = mybir.ActivationFunctionType.Exp
    exp2_op = _register_exp2_op() if dve_every else None

    nc = bacc.Bacc("TRN2", target_bir_lowering=False, debug=False,
                   num_devices=N_CORES)

    kq = nc.dram_tensor("kq", [2, 2, 128, SQ], f32r, kind="ExternalInput").ap()
    vv = nc.dram_tensor("vv", [2, 128, NT * 2 * VW], bf16,
                        kind="ExternalInput").ap()
    outU = nc.dram_tensor("outU", [2, VW, 2 * SQ], f32,
                          kind="ExternalOutput").ap()

    with tile.TileContext(nc) as tc, ExitStack() as ctx:
        in_pool = ctx.enter_context(tc.tile_pool(name="ins", bufs=1))
        ex_pool = ctx.enter_context(tc.tile_pool(name="ex", bufs=4))
        v1_pool = ctx.enter_context(tc.tile_pool(name="v1", bufs=3))
        exi_pool = ctx.enter_context(tc.tile_pool(name="exi", bufs=4))
        cts_pool = ctx.enter_context(tc.tile_pool(name="cts", bufs=2))
        # PSUM: sc 2 bufs x 2 banks + ct 2 bufs x 2 banks = 8 banks
        ps_sc = ctx.enter_context(tc.tile_pool(name="ps_sc", bufs=2, space="PSUM"))
        ps_ct = ctx.enter_context(tc.tile_pool(name="ps_ct", bufs=2, space="PSUM"))

        # --- input tiles + ordered DMA ---------------------------------
        kt = [in_pool.tile([128, SQ], f32r, tag=f"kt{g}", name=f"kt{g}")
              for g in range(2)]
        qt = [in_pool.tile([128, SQ], f32r, tag=f"qt{g}", name=f"qt{g}")
              for g in range(2)]
        vt = [in_pool.tile([128, NT * 2 * VW], bf16, tag=f"vt{g}", name=f"vt{g}")
              for g in range(2)]
        VH = 8 * 2 * VW          # half of the V' columns (j-blocks 0-7)
        # pair 0, consumption order
        QH = 4 * 2 * VW          # V' columns for j-blocks 0-3
        nc.sync.dma_start(out=kt[0][:, 0:512], in_=kq[0, 0, :, 0:512])
        nc.sync.dma_start(out=qt[0][:, 0:512], in_=kq[0, 1, :, 0:512])
        nc.sync.dma_start(out=vt[0][:, 0:QH], in_=vv[0, :, 0:QH])
        nc.sync.dma_start(out=kt[0][:, 512:1024], in_=kq[0, 0, :, 512:1024])
        nc.sync.dma_start(out=vt[0][:, QH:VH], in_=vv[0, :, QH:VH])
        nc.sync.dma_start(out=kt[0][:, 1024:2048], in_=kq[0, 0, :, 1024:2048])
        nc.sync.dma_start(out=vt[0][:, VH:2 * VH], in_=vv[0, :, VH:2 * VH])
        nc.sync.dma_start(out=qt[0][:, 512:2048], in_=kq[0, 1, :, 512:2048])
        # pair 1
        nc.sync.dma_start(out=kt[1][:], in_=kq[1, 0])
        nc.sync.dma_start(out=qt[1][:], in_=kq[1, 1])
        nc.sync.dma_start(out=vt[1][:], in_=vv[1])

        v3 = [vt[g].rearrange("p (j c) -> p j c", c=2 * VW) for g in range(2)]

        steps = [(g, c, j) for g in range(2) for c in range(NCH)
                 for j in range(NT)]

        def emit_qk(s):
            g, c, j = steps[s]
            s0 = c * SCH
            sc = ps_sc.tile([128, 1024], f32, tag="sc", name="sc")
            nc.tensor.matmul(
                sc[:, 0:512],
                lhsT=kt[g][0:64, j * 128:(j + 1) * 128],
                rhs=qt[g][0:64, s0:s0 + SCH],
                start=True, stop=True)
            nc.tensor.matmul(
                sc[:, 512:1024],
                lhsT=kt[g][64:128, j * 128:(j + 1) * 128],
                rhs=qt[g][64:128, s0:s0 + SCH],
                start=True, stop=True)
            return sc

        CT = [None]

        def emit_pv(s, ex):
            g, c, j = steps[s]
            if j == 0:
                CT[0] = ps_ct.tile([128, 1024], f32, tag="ct", name="ct")
            nc.tensor.matmul(
                CT[0][0:VW, 0:512],
                lhsT=v3[g][:, j, 0:VW],
                rhs=ex[:, 0:512],
                start=(j == 0), stop=(j == NT - 1))
            nc.tensor.matmul(
                CT[0][0:VW, 512:1024],
                lhsT=v3[g][:, j, VW:2 * VW],
                rhs=ex[:, 512:1024],
                start=(j == 0), stop=(j == NT - 1))

        def emit_tail(s):
            g, c, j = steps[s]
            if j != NT - 1:
                return
            s0 = c * SCH
            cts = cts_pool.tile([VW, 1024], f32, tag="cts", name="cts")
            nc.scalar.copy(cts[:], CT[0][0:VW, :])
            nc.sync.dma_start(out=outU[g, :, s0:s0 + SCH],
                              in_=cts[:, 0:512])
            nc.scalar.dma_start(out=outU[g, :, SQ + s0:SQ + s0 + SCH],
                                in_=cts[:, 512:1024])

        # Software pipeline: exp chases QK immediately; PV trails by LAG
        # steps so the DVE+GPSIMD exp chain latency stays off the PE's
        # in-order critical path.
        LAG = 3
        pvq = []
        sc_cur = emit_qk(0)
        for s in range(len(steps) + LAG):
            if s < len(steps):
                sc_next = emit_qk(s + 1) if s + 1 < len(steps) else None
                if dve_every and s % dve_every == dve_every - 1:
                    v1 = v1_pool.tile([128, 1024], f32, tag="v1", name="v1")
                    nc.vector._custom_dve(
                        exp2_op, out=v1[:], in0=sc_cur[:],
                        s0=EXP_C0, s1=EXP_C1, imm2=EXP_B2)
                    exi = exi_pool.tile([128, 1024], i16, tag="exi",
                                        name="exi")
                    nc.gpsimd.tensor_scalar(
                        exi[:], v1[:], EXP_BETA0, 2.0**7,
                        op0=mybir.AluOpType.add, op1=mybir.AluOpType.mult)
                    ex = exi.bitcast(bf16)
                else:
                    ex = ex_pool.tile([128, 1024], bf16, tag="ex", name="ex")
                    nc.scalar.activation(ex[:], sc_cur[:], EXP, scale=LN2)
                pvq.append((s, ex))
                sc_cur = sc_next
            if s >= LAG:
                sp, exp_tile = pvq.pop(0)
                emit_pv(sp, exp_tile)
                emit_tail(sp)
    nc.compile()
    return nc


def get_nc(dve_every=2):
    key = ("nc", dve_every)
    if key not in _CACHE:
        _CACHE[key] = _build_program(dve_every)
    return _CACHE[key]


def make_in_maps(query, key, value):
    """Host-side sharding + layout prep. Returns list of per-core input maps."""
    query = np.asarray(query, dtype=np.float32) * np.float32(LOG2E)
    key = np.asarray(key, dtype=np.float32)
    value = np.asarray(value, dtype=np.float32)
    in_maps = []
    for c in range(N_CORES):
        b = c // 4
        n0 = HEADS_PER_CORE * (c % 4)
        q = query[:, b, n0:n0 + 4, :]   # [2048, 4, 64]
        k = key[:, b, n0:n0 + 4, :]
        v = value[:, b, n0:n0 + 4, :]
        qt = _round_fp32r(q.transpose(1, 2, 0).reshape(2, 128, SQ))
        kt = _round_fp32r(k.transpose(1, 2, 0).reshape(2, 128, SQ))
        kq = np.ascontiguousarray(np.stack([kt, qt], axis=1))  # [2,2,128,SQ]
        vp = np.concatenate(
            [v, np.ones((SQ, 4, 1), np.float32),
             np.zeros((SQ, 4, 1), np.float32)], axis=2)
        vp = vp.reshape(16, 128, 2, 2 * VW).transpose(2, 1, 0, 3)
        import ml_dtypes
        vp = np.ascontiguousarray(
            vp.reshape(2, 128, NT * 2 * VW)).astype(ml_dtypes.bfloat16)
        in_maps.append({"kq": kq, "vv": vp})
    return in_maps


def postprocess_core(outU):
    """outU [2, 66, 4096] -> normalized per-core output [2048, 4, 64]."""
    outU = np.asarray(outU)
    res = np.empty((SQ, 4, HN), np.float32)
    for g in range(2):
        for h in range(2):
            blk = outU[g, :, h * SQ:(h + 1) * SQ]
            ctx = blk[0:64, :]
            den = blk[64, :]
            res[:, 2 * g + h, :] = (ctx / den).T
    return res


def assemble_output(results):
    out = np.empty((SQ, B, NHEADS, HN), np.float32)
    for c in range(N_CORES):
        b = c // 4
        n0 = HEADS_PER_CORE * (c % 4)
        out[:, b, n0:n0 + 4, :] = postprocess_core(results[c]["outU"])
    return out.reshape(SQ, B, NHEADS * HN)


def kernel(query, key, value):
    try:
        from concourse.bass_utils import run_bass_kernel_spmd
    except ImportError:
        import sys
        sys.path.insert(0, "/opt/trn_rl_repo")
        from concourse.bass_utils import run_bass_kernel_spmd

    nc = get_nc()
    in_maps = make_in_maps(query, key, value)
    res = run_bass_kernel_spmd(nc, in_maps, list(range(N_CORES)))
    return assemble_output(res.results)


# revision 15
# speedup vs baseline: 1.0136x; 1.0136x over previous
"""Trainium2 Bass kernel for unscaled dot-product attention.

Shapes (hardcoded): query/key/value [2048, 2, 16, 64] fp32.
  scores = einsum('sbnh,tbnh->bnst', q, k)   (UNscaled)
  probs  = softmax(scores, axis=-1)
  out    = einsum('bnst,tbnh->sbnh', probs, v).reshape(2048, 2, 1024)

Sharding: the 32 (b, n) head-slices are split 4-per-core across 8 cores
(core c -> b = c//4, heads 4*(c%4) .. +4). Each core computes attention
for its 4 heads independently; no cross-device communication.

Device-side strategy (per core, heads processed in 2 pairs):
  - Inputs arrive as separate K^T / Q^T / V' DRAM tensors, DMA-ed in
    consumption order (first K/Q/V chunks of pair 0 first) so the first
    QK matmul can start as soon as ~1.8MB has landed instead of waiting
    for the full 6.4MB.
  - Q is pre-scaled by log2(e) on the host, so on-device scores are
    t = score*log2(e); exp is computed as 2^t (ACT runs Exp with
    scale=ln2, the custom DVE path computes 2^t directly).
  - scores are computed TRANSPOSED per 128-t block with two heads packed
    into the 128 PE contraction rows (row-tiled concurrent matmuls).
  - Normalization happens ON THE HOST: the device ships the unnormalized
    context^T (PV accumulation, with a ones column producing the softmax
    denominator in row 64) straight from PSUM to DRAM; the host divides
    and transposes. This removes reciprocal/broadcast/multiply work from
    the device entirely.
  - exp() is split between the Scalar engine (ACT spline Exp) and the
    Vector engine (custom 8-stage DVE op computing the 2^t bit pattern:
    magic-rounding range reduction + quadratic mantissa correction,
    finished by a GPSIMD affine pass + f32->int32 convert that builds
    the final float bits).
"""

import numpy as np

SQ, B, NHEADS, HN = 2048, 2, 16, 64
N_CORES = 8
HEADS_PER_CORE = 4
VW = 66                     # V' columns per head (64 V + ones + pad)
LOG2E = 1.4426950408889634
LN2 = 0.6931471805599453

SCH = 512                   # s-chunk per inner loop
NCH = SQ // SCH             # 4
NT = SQ // 128              # 16 t-blocks

# custom-DVE exp2 pass-1 constants (see _register_exp2_op)
EXP_C0 = 126.5
EXP_C1 = 1.5 * 2.0**23
EXP_B2 = 0.333205057        # minimax quadratic coeff, lambda=1 kink-free
# pass-2 bias (centered for truncation; a round-mode instead just shifts the
# global scale, which softmax normalization cancels)
EXP_BETA0 = 0.415565974 + 0.5 / 128

_CACHE = {}


def _register_exp2_op():
    """Register the EXP2_PASS1_ANT custom DVE op (runtime extension of
    concourse.dve_ops via its documented OPS registry).

    Computes, per element (t = score * log2(e), from Src0):
      i+127 = magic-round(t + 126.5)      (floor(t) + 127)
      h     = frac(t) - 0.5
      v1    = (i + 127) + h + b2*h^2      (8 ALU stages exactly)
    A second pass ((v1 + beta0) * 2^23 -> int32) then forms the IEEE-754
    bit pattern of ~2^t.  Max relative error ~2.9e-3, global scale 1
    (cancels in softmax normalization anyway).
    """
    import concourse.dve_ops as dve_ops
    for op in dve_ops.OPS:
        if op.name == "EXP2_PASS1_ANT":
            return op
    from concourse.dve_spec import Spec, Src0, C0, C1, C2, One, lower
    from concourse.dve_spec import _has_src1
    from concourse.dve_uop import DveOpSpec

    pre = Src0 + C0
    u = pre + C1
    w = u - C1
    s = pre - w
    h2 = (s * C2) + One
    g = s * h2
    body = w + g

    def ref(in0, in1, s0, s1, imm2):
        f32 = np.float32
        t = np.asarray(in0, f32)
        pre = (t + f32(s0)).astype(f32)
        u = (pre + f32(s1)).astype(f32)
        w = (u - f32(s1)).astype(f32)
        sh = (pre - w).astype(f32)
        hh2 = ((sh * f32(imm2)).astype(f32) + f32(1.0)).astype(f32)
        g = (sh * hh2).astype(f32)
        return (w + g).astype(f32)

    spec = Spec(body=body, reference=ref)
    row = dve_ops._CUSTOM_DVE_ROW_BASE + len(dve_ops.OPS)
    sha = {}
    for ver in ("v3", "v4"):
        sha[ver] = DveOpSpec(
            name="EXP2_PASS1_ANT", opcode=row, uops=lower(spec, ver=ver),
            rd1_en=_has_src1(spec)).sha(ver)
    op = dve_ops.DveOp("EXP2_PASS1_ANT", spec, subdim=False, uops_sha=sha)
    dve_ops.OPS.append(op)
    dve_ops._SUB_OPCODE_FOR_NAME[op.name] = row
    dve_ops.CUSTOM_DVE_SPECS[op.name] = spec
    return op


def _round_fp32r(x):
    """Round fp32 array to the fp32r grid (11 explicit mantissa bits,
    round-to-nearest-even, low 12 bits zero)."""
    u = np.ascontiguousarray(x, np.float32).view(np.uint32)
    lsb = (u >> 12) & 1
    u = (u + 0x7FF + lsb) & 0xFFFFF000
    return u.astype(np.uint32).view(np.float32)


def _build_program(dve_every=0):
    """dve_every=0: all exp on ACT.  dve_every=k>0: every k-th step's exp
    runs on the DVE+GPSIMD path instead."""
    from contextlib import ExitStack

    import concourse.bacc as bacc
    import concourse.mybir as mybir
    import concourse.tile as tile

    f32 = mybir.dt.float32
    f32r = mybir.dt.float32r
    bf16 = mybir.dt.bfloat16
    i16 = mybir.dt.int16
    EXP = mybir.ActivationFunctionType.Exp
    exp2_op = _register_exp2_op() if dve_every else None

    nc = bacc.Bacc("TRN2", target_bir_lowering=False, debug=False,
                   num_devices=N_CORES)

    kq = nc.dram_tensor("kq", [2, 2, 128, SQ], f32r, kind="ExternalInput").ap()
    vv = nc.dram_tensor("vv", [2, 128, NT * 2 * VW], bf16,
                        kind="ExternalInput").ap()
    outU = nc.dram_tensor("outU", [2, VW, 2 * SQ], f32,
                          kind="ExternalOutput").ap()

    with tile.TileContext(nc) as tc, ExitStack() as ctx:
        in_pool = ctx.enter_context(tc.tile_pool(name="ins", bufs=1))
        ex_pool = ctx.enter_context(tc.tile_pool(name="ex", bufs=4))
        v1_pool = ctx.enter_context(tc.tile_pool(name="v1", bufs=3))
        exi_pool = ctx.enter_context(tc.tile_pool(name="exi", bufs=4))
        cts_pool = ctx.enter_context(tc.tile_pool(name="cts", bufs=2))
        # PSUM: sc 2 bufs x 2 banks + ct 2 bufs x 2 banks = 8 banks
        ps_sc = ctx.enter_context(tc.tile_pool(name="ps_sc", bufs=2, space="PSUM"))
        ps_ct = ctx.enter_context(tc.tile_pool(name="ps_ct", bufs=2, space="PSUM"))

        # --- input tiles + ordered DMA ---------------------------------
        kt = [in_pool.tile([128, SQ], f32r, tag=f"kt{g}", name=f"kt{g}")
              for g in range(2)]
        qt = [in_pool.tile([128, SQ], f32r, tag=f"qt{g}", name=f"qt{g}")
              for g in range(2)]
        vt = [in_pool.tile([128, NT * 2 * VW], bf16, tag=f"vt{g}", name=f"vt{g}")
              for g in range(2)]
        VH = 8 * 2 * VW          # half of the V' columns (j-blocks 0-7)
        # pair 0, consumption order
        QH = 4 * 2 * VW          # V' columns for j-blocks 0-3
        nc.sync.dma_start(out=kt[0][:, 0:512], in_=kq[0, 0, :, 0:512])
        nc.sync.dma_start(out=qt[0][:, 0:1024], in_=kq[0, 1, :, 0:1024])
        nc.sync.dma_start(out=vt[0][:, 0:QH], in_=vv[0, :, 0:QH])
        nc.sync.dma_start(out=kt[0][:, 512:1024], in_=kq[0, 0, :, 512:1024])
        nc.sync.dma_start(out=vt[0][:, QH:VH], in_=vv[0, :, QH:VH])
        nc.sync.dma_start(out=kt[0][:, 1024:2048], in_=kq[0, 0, :, 1024:2048])
        nc.sync.dma_start(out=vt[0][:, VH:2 * VH], in_=vv[0, :, VH:2 * VH])
        nc.sync.dma_start(out=qt[0][:, 1024:2048], in_=kq[0, 1, :, 1024:2048])
        # pair 1
        nc.sync.dma_start(out=kt[1][:], in_=kq[1, 0])
        nc.sync.dma_start(out=qt[1][:], in_=kq[1, 1])
        nc.sync.dma_start(out=vt[1][:], in_=vv[1])

        v3 = [vt[g].rearrange("p (j c) -> p j c", c=2 * VW) for g in range(2)]

        # Loop order (g, c-pair, j, c-inner): consecutive steps share the
        # QK j-block weights, so the PE can keep the stationary operand
        # across two matmul pairs; both chunks of a pair accumulate into
        # separate live CT tiles.
        steps = [(g, 2 * c2 + ci, j) for g in range(2) for c2 in range(2)
                 for j in range(NT) for ci in range(2)]

        def emit_qk(s):
            g, c, j = steps[s]
            s0 = c * SCH
            sc = ps_sc.tile([128, 1024], f32, tag="sc", name="sc")
            nc.tensor.matmul(
                sc[:, 0:512],
                lhsT=kt[g][0:64, j * 128:(j + 1) * 128],
                rhs=qt[g][0:64, s0:s0 + SCH],
                start=True, stop=True)
            nc.tensor.matmul(
                sc[:, 512:1024],
                lhsT=kt[g][64:128, j * 128:(j + 1) * 128],
                rhs=qt[g][64:128, s0:s0 + SCH],
                start=True, stop=True)
            return sc

        CT = {}

        def emit_pv(s, ex):
            g, c, j = steps[s]
            if j == 0:
                CT[c % 2] = ps_ct.tile([128, 1024], f32, tag="ct", name="ct")
            ct = CT[c % 2]
            nc.tensor.matmul(
                ct[0:VW, 0:512],
                lhsT=v3[g][:, j, 0:VW],
                rhs=ex[:, 0:512],
                start=(j == 0), stop=(j == NT - 1))
            nc.tensor.matmul(
                ct[0:VW, 512:1024],
                lhsT=v3[g][:, j, VW:2 * VW],
                rhs=ex[:, 512:1024],
                start=(j == 0), stop=(j == NT - 1))

        def emit_tail(s):
            g, c, j = steps[s]
            if j != NT - 1:
                return
            s0 = c * SCH
            cts = cts_pool.tile([VW, 1024], f32, tag="cts", name="cts")
            nc.scalar.copy(cts[:], CT[c % 2][0:VW, :])
            nc.sync.dma_start(out=outU[g, :, s0:s0 + SCH],
                              in_=cts[:, 0:512])
            nc.scalar.dma_start(out=outU[g, :, SQ + s0:SQ + s0 + SCH],
                                in_=cts[:, 512:1024])

        # Software pipeline: exp chases QK immediately; PV trails by LAG
        # steps so the DVE+GPSIMD exp chain latency stays off the PE's
        # in-order critical path.
        LAG = 3
        pvq = []
        sc_cur = emit_qk(0)
        for s in range(len(steps) + LAG):
            if s < len(steps):
                sc_next = emit_qk(s + 1) if s + 1 < len(steps) else None
                if dve_every and s % dve_every == dve_every - 1:
                    v1 = v1_pool.tile([128, 1024], f32, tag="v1", name="v1")
                    nc.vector._custom_dve(
                        exp2_op, out=v1[:], in0=sc_cur[:],
                        s0=EXP_C0, s1=EXP_C1, imm2=EXP_B2)
                    exi = exi_pool.tile([128, 1024], i16, tag="exi",
                                        name="exi")
                    nc.gpsimd.tensor_scalar(
                        exi[:], v1[:], EXP_BETA0, 2.0**7,
                        op0=mybir.AluOpType.add, op1=mybir.AluOpType.mult)
                    ex = exi.bitcast(bf16)
                else:
                    ex = ex_pool.tile([128, 1024], bf16, tag="ex", name="ex")
                    nc.scalar.activation(ex[:], sc_cur[:], EXP, scale=LN2)
                pvq.append((s, ex))
                sc_cur = sc_next
            if s >= LAG:
                sp, exp_tile = pvq.pop(0)
                emit_pv(sp, exp_tile)
                emit_tail(sp)
    nc.compile()
    return nc


def get_nc(dve_every=2):
    key = ("nc", dve_every)
    if key not in _CACHE:
        _CACHE[key] = _build_program(dve_every)
    return _CACHE[key]


def make_in_maps(query, key, value):
    """Host-side sharding + layout prep. Returns list of per-core input maps."""
    query = np.asarray(query, dtype=np.float32) * np.float32(LOG2E)
    key = np.asarray(key, dtype=np.float32)
    value = np.asarray(value, dtype=np.float32)
    in_maps = []
    for c in range(N_CORES):
        b = c // 4
        n0 = HEADS_PER_CORE * (c % 4)
        q = query[:, b, n0:n0 + 4, :]   # [2048, 4, 64]
        k = key[:, b, n0:n0 + 4, :]
        v = value[:, b, n0:n0 + 4, :]
        qt = _round_fp32r(q.transpose(1, 2, 0).reshape(2, 128, SQ))
        kt = _round_fp32r(k.transpose(1, 2, 0).reshape(2, 128, SQ))
        kq = np.ascontiguousarray(np.stack([kt, qt], axis=1))  # [2,2,128,SQ]
        vp = np.concatenate(
            [v, np.ones((SQ, 4, 1), np.float32),
             np.zeros((SQ, 4, 1), np.float32)], axis=2)
        vp = vp.reshape(16, 128, 2, 2 * VW).transpose(2, 1, 0, 3)
        import ml_dtypes
        vp = np.ascontiguousarray(
            vp.reshape(2, 128, NT * 2 * VW)).astype(ml_dtypes.bfloat16)
        in_maps.append({"kq": kq, "vv": vp})
    return in_maps


def postprocess_core(outU):
    """outU [2, 66, 4096] -> normalized per-core output [2048, 4, 64]."""
    outU = np.asarray(outU)
    res = np.empty((SQ, 4, HN), np.float32)
    for g in range(2):
        for h in range(2):
            blk = outU[g, :, h * SQ:(h + 1) * SQ]
            ctx = blk[0:64, :]
            den = blk[64, :]
            res[:, 2 * g + h, :] = (ctx / den).T
    return res


def assemble_output(results):
    out = np.empty((SQ, B, NHEADS, HN), np.float32)
    for c in range(N_CORES):
        b = c // 4
        n0 = HEADS_PER_CORE * (c % 4)
        out[:, b, n0:n0 + 4, :] = postprocess_core(results[c]["outU"])
    return out.reshape(SQ, B, NHEADS * HN)


def kernel(query, key, value):
    try:
        from concourse.bass_utils import run_bass_kernel_spmd
    except ImportError:
        import sys
        sys.path.insert(0, "/opt/trn_rl_repo")
        from concourse.bass_utils import run_bass_kernel_spmd

    nc = get_nc()
    in_maps = make_in_maps(query, key, value)
    res = run_bass_kernel_spmd(nc, in_maps, list(range(N_CORES)))
    return assemble_output(res.results)
